# revision 22
# baseline (speedup 1.0000x reference)
"""Trainium2 Bass kernel for nn_Dilate: 5x5 max-filter (cv2.dilate) over
(64, 384, 384, 3) fp32 images, SAME padding, output (64, 384, 384, 3, 1).

Sharding: pure batch data-parallel, 8 images per NeuronCore. Per core
the workload is [3072 rows, 1152 cols] fp32 (rows = 8 images x 384 H;
cols = 384 W x 3 C interleaved). Partition p (0..127) owns 24
consecutive rows [24p, 24p+24). The host pre-tiles each core's input
into [128 partitions, 28 rows, 1152] — the 24 owned rows plus a 2-row
halo on each side, edge-clamped at image boundaries (for max-pooling,
clamp padding == SAME padding: max(r0,r0,r0,r1,r2) == max(r0,r1,r2)) —
the canonical halo-exchange stencil sharding, so the device sees one
uniform DMA stream with no boundary special cases.

fp16 pipeline (the DVE TensorTensor(max) only gets its 2x perf mode
with a packed 2-byte dtype, so fp32 compute would be twice as slow):
  1. DMA fp32 row-chunks into the staging tile S[128, 28, 1152].
  2. ScalarE (ACT) converts S -> fp16 working tile W[128, 28, 1164]
     (6-elem zero pad per side; inputs are >= 0 so max with 0 is the
     identity for the horizontal SAME padding).
  3. DVE runs the separable 5x5 max as 6 shifted in-place
     tensor_tensor(max) passes over W, all along the free axis:
       vertical:   win2 -> win3 -> win5 over rows   (shifts +1, +1, +2)
       horizontal: win2 -> win3 -> win5 over pixels (shifts +3, +3, +6)
     Each in-place op only reads *ahead* of what it writes, which is
     safe on the DVE's streaming pipeline.
  4. DMA W rows back to HBM as fp16; the host converts to fp32.
The stages are software-pipelined in small row-steps (V1 leads, V2 lags
one step, V3/H/store lag two) so DMA, ACT and DVE overlap; DVE is the
bottleneck engine (~92us busy of ~105us total per core).
"""

import numpy as np


def _ensure_path():
    try:
        import concourse  # noqa: F401
    except ImportError:
        import sys

        for p in ("/opt/trn_rl_repo", "/root/.axon_site/_ro/trn_rl_repo"):
            if p not in sys.path:
                sys.path.insert(0, p)


N_CORES = 8
B_PER = 8  # images per core
H = 384
W = 384
C = 3
WROW = W * C  # 1152
ROWS = B_PER * H  # 3072 rows per core
RP = ROWS // 128  # 24 rows per partition
PAD = 6  # 2 pixels * 3 channels zero pad each side
PADW = WROW + 2 * PAD  # 1164
HR = RP + 4  # 28 rows incl. 2-row halo on each side
W0 = PAD
W1 = PAD + WROW

# input DMA chunks over S rows [0, 28) (S row r = input row 24p+r-2,
# edge-clamped; the halo is pre-tiled on the host)
DMA_CHUNKS = [
    (0, 2), (2, 3), (3, 4), (4, 5), (5, 7), (7, 9),
    (9, 12), (12, 15), (15, 18), (18, 21), (21, 24), (24, 28),
]
# ACT fp32->fp16 convert chunks (finer than DMA at the front so the
# first rows reach the DVE with minimal chunk latency)
ACT_CHUNKS = [
    (0, 1), (1, 2), (2, 3), (3, 4), (4, 5), (5, 7), (7, 9),
    (9, 12), (12, 15), (15, 18), (18, 21), (21, 24), (24, 28),
]
# DVE pipeline steps: step i advances V1 to steps[i]; V2 advances to
# steps[i-1]-1 and V3/H/store to steps[i-2]-3 (clamped), so every pass
# only reads rows finished in *prior* steps and the step's only stall
# point is the trailing V1 advance.
V1_STEPS = [1, 2, 3, 4, 6, 8, 11, 14, 17, 20, 23, 25, 26, 27, 28, 28]
V1_MAX, V2_MAX, V3_MAX = HR - 1, HR - 2, RP  # 27, 26, 24

_CACHE = {}


def _build_nc(v1_steps=None, dma_chunks=None, act_chunks=None, dve_conv=0):
    _ensure_path()
    from concourse import bacc, mybir, tile
    from concourse.ap import AP

    f32 = mybir.dt.float32
    f16 = mybir.dt.float16
    steps = list(v1_steps or V1_STEPS)
    dchunks = list(dma_chunks or DMA_CHUNKS)
    # the DVE converts rows [0, dve_conv) itself (it is idle during the
    # ramp anyway, and skipping the ACT hop cuts the feed latency); ACT
    # handles the rest.
    achunks = [
        (max(r0, dve_conv), r1)
        for r0, r1 in (act_chunks or ACT_CHUNKS)
        if r1 > dve_conv
    ]
    assert steps[-1] >= V3_MAX + 4

    nc = bacc.Bacc(
        "TRN2",
        target_bir_lowering=False,
        debug=False,
        enable_asserts=False,
        num_devices=N_CORES,
    )
    x = nc.dram_tensor("x", [128, HR, WROW], f32, kind="ExternalInput")
    y = nc.dram_tensor("y", [ROWS, WROW], f16, kind="ExternalOutput")

    with tile.TileContext(nc) as tc:
        with tc.tile_pool(name="pool", bufs=1) as pool:
            s = pool.tile([128, HR, WROW], f32, name="s", tag="s")
            w = pool.tile([128, HR, PADW], f16, name="w", tag="w")

            # warm the ACT activation table at t=0 so the implicit
            # ACT_TABLE_LOAD isn't charged to the first convert. Touches
            # only pad cols; the memsets below re-zero them.
            nc.scalar.copy(w[:, 0:1, 0:2], w[:, 0:1, 0:2])
            # zero the side pads (idle Pool engine, off the critical path)
            nc.gpsimd.memset(w[:, :, 0:PAD], 0.0)
            nc.gpsimd.memset(w[:, :, W1:PADW], 0.0)

            # ---- input DMA (uniform chunk stream, halo pre-tiled) ----
            for r0, r1 in dchunks:
                nc.sync.dma_start(
                    s[:, r0:r1, :],
                    AP(
                        x,
                        r0 * WROW,
                        [[HR * WROW, 128], [WROW, r1 - r0], [1, WROW]],
                    ),
                )

            # ---- ACT fp32 -> fp16 convert ----
            for r0, r1 in achunks:
                nc.scalar.copy(w[:, r0:r1, W0:W1], s[:, r0:r1, :])

            # ---- DVE passes + stores, software-pipelined ----
            e = nc.vector
            f1 = f2 = f3 = 0  # frontiers: rows done per pass
            fc = 0  # DVE-convert frontier (rows < dve_conv)

            def vpass(a0, a1, shift):
                if a1 > a0:
                    e.tensor_max(
                        w[:, a0:a1, W0:W1],
                        w[:, a0:a1, W0:W1],
                        w[:, a0 + shift : a1 + shift, W0:W1],
                    )

            for i, a in enumerate(steps):
                n1 = min(a, V1_MAX)
                n2 = min(max(steps[i - 1] - 1, 0), V2_MAX) if i >= 1 else 0
                n3 = min(max(steps[i - 2] - 3, 0), V3_MAX) if i >= 2 else 0
                assert n2 + 1 <= f1 or n2 <= f2, (i, n2, f1)
                assert n3 + 2 <= f2 or n3 <= f3, (i, n3, f2)
                vpass(f2, n2, 1)  # win3 over rows
                vpass(f3, n3, 2)  # win5
                if n3 > f3:
                    r0, r1 = f3, n3
                    # horizontal win2/win3/win5 (pixel stride = C = 3)
                    e.tensor_max(
                        w[:, r0:r1, 0 : PADW - 3],
                        w[:, r0:r1, 0 : PADW - 3],
                        w[:, r0:r1, 3:PADW],
                    )
                    e.tensor_max(
                        w[:, r0:r1, 0 : PADW - 6],
                        w[:, r0:r1, 0 : PADW - 6],
                        w[:, r0:r1, 3 : PADW - 3],
                    )
                    e.tensor_max(
                        w[:, r0:r1, 0:WROW],
                        w[:, r0:r1, 0:WROW],
                        w[:, r0:r1, PAD : PAD + WROW],
                    )
                    nc.sync.dma_start(
                        AP(
                            y,
                            r0 * WROW,
                            [[RP * WROW, 128], [WROW, r1 - r0], [1, WROW]],
                        ),
                        w[:, r0:r1, 0:WROW],
                    )
                need = min(n1 + 1, dve_conv)
                if need > fc:
                    # fp32 -> fp16 convert on the DVE (tensor_scalar max
                    # with 0.0: identity for data >= 0, runs in 2x_2p mode)
                    e.tensor_scalar_max(
                        w[:, fc:need, W0:W1], s[:, fc:need, :], 0.0
                    )
                    fc = need
                vpass(f1, n1, 1)  # win2 over rows (stalls on ACT, so last)
                f1, f2, f3 = n1, n2, n3
            assert (f1, f2, f3) == (V1_MAX, V2_MAX, V3_MAX)

    nc.compile()
    return nc


def _get_nc():
    if "nc" not in _CACHE:
        _CACHE["nc"] = _build_nc()
    return _CACHE["nc"]


def _row_index():
    # IDX[p, r] = input row (within a core's [3072, 1152] view) whose data
    # partition p's staging row r holds: 24p + r - 2, edge-clamped to the
    # owning image's row range (replicated edge row == SAME pad for max).
    p = np.arange(128)[:, None]
    r = np.arange(HR)[None, :]
    r_abs = RP * p + r - 2
    img_lo = (p // 16) * H
    return np.clip(r_abs, img_lo, img_lo + H - 1)


def _run(images, trace=False):
    _ensure_path()
    from concourse import bass_utils

    images = np.ascontiguousarray(np.asarray(images, dtype=np.float32))
    assert images.shape == (N_CORES * B_PER, H, W, C), images.shape
    nc = _get_nc()
    per_core = images.reshape(N_CORES, ROWS, WROW)
    idx = _row_index()
    in_maps = [
        {"x": np.ascontiguousarray(per_core[i][idx])} for i in range(N_CORES)
    ]
    res = bass_utils.run_bass_kernel_spmd(
        nc, in_maps, core_ids=list(range(N_CORES)), trace=trace
    )
    out = np.concatenate(
        [res.results[i]["y"].astype(np.float32) for i in range(N_CORES)], axis=0
    )
    out = out.reshape(N_CORES * B_PER, H, W, C)[..., None]
    return out, res


def kernel(images, k=None):
    out, _ = _run(images, trace=False)
    return out


# revision 23
# speedup vs baseline: 1.0024x; 1.0024x over previous
"""Trainium2 Bass kernel for nn_Dilate: 5x5 max-filter (cv2.dilate) over
(64, 384, 384, 3) fp32 images, SAME padding, output (64, 384, 384, 3, 1).

Sharding: pure batch data-parallel, 8 images per NeuronCore. Per core
the workload is [3072 rows, 1152 cols] fp32 (rows = 8 images x 384 H;
cols = 384 W x 3 C interleaved). Partition p (0..127) owns 24
consecutive rows [24p, 24p+24). The host pre-tiles each core's input
into [128 partitions, 28 rows, 1152] — the 24 owned rows plus a 2-row
halo on each side, edge-clamped at image boundaries (for max-pooling,
clamp padding == SAME padding: max(r0,r0,r0,r1,r2) == max(r0,r1,r2)) —
the canonical halo-exchange stencil sharding, so the device sees one
uniform DMA stream with no boundary special cases.

fp16 pipeline (the DVE TensorTensor(max) only gets its 2x perf mode
with a packed 2-byte dtype, so fp32 compute would be twice as slow):
  1. DMA fp32 row-chunks into the staging tile S[128, 28, 1152].
  2. ScalarE (ACT) converts S -> fp16 working tile W[128, 28, 1164]
     (6-elem zero pad per side; inputs are >= 0 so max with 0 is the
     identity for the horizontal SAME padding).
  3. DVE runs the separable 5x5 max as 6 shifted in-place
     tensor_tensor(max) passes over W, all along the free axis:
       vertical:   win2 -> win3 -> win5 over rows   (shifts +1, +1, +2)
       horizontal: win2 -> win3 -> win5 over pixels (shifts +3, +3, +6)
     Each in-place op only reads *ahead* of what it writes, which is
     safe on the DVE's streaming pipeline.
  4. DMA W rows back to HBM as fp16; the host converts to fp32.
The stages are software-pipelined in small row-steps (V1 leads, V2 lags
one step, V3/H/store lag two) so DMA, ACT and DVE overlap; DVE is the
bottleneck engine (~92us busy of ~105us total per core).
"""

import numpy as np


def _ensure_path():
    try:
        import concourse  # noqa: F401
    except ImportError:
        import sys

        for p in ("/opt/trn_rl_repo", "/root/.axon_site/_ro/trn_rl_repo"):
            if p not in sys.path:
                sys.path.insert(0, p)


N_CORES = 8
B_PER = 8  # images per core
H = 384
W = 384
C = 3
WROW = W * C  # 1152
ROWS = B_PER * H  # 3072 rows per core
RP = ROWS // 128  # 24 rows per partition
PAD = 6  # 2 pixels * 3 channels zero pad each side
PADW = WROW + 2 * PAD  # 1164
HR = RP + 4  # 28 rows incl. 2-row halo on each side
W0 = PAD
W1 = PAD + WROW

# input DMA chunks over S rows [0, 28) (S row r = input row 24p+r-2,
# edge-clamped; the halo is pre-tiled on the host)
DMA_CHUNKS = [
    (0, 1), (1, 2), (2, 3), (3, 4), (4, 5), (5, 7), (7, 9),
    (9, 12), (12, 15), (15, 18), (18, 21), (21, 24), (24, 28),
]
# ACT fp32->fp16 convert chunks (finer than DMA at the front so the
# first rows reach the DVE with minimal chunk latency)
ACT_CHUNKS = [
    (0, 1), (1, 2), (2, 3), (3, 4), (4, 5), (5, 7), (7, 9),
    (9, 12), (12, 15), (15, 18), (18, 21), (21, 24), (24, 28),
]
# DVE pipeline steps: step i advances V1 to steps[i]; V2 advances to
# steps[i-1]-1 and V3/H/store to steps[i-2]-3 (clamped), so every pass
# only reads rows finished in *prior* steps and the step's only stall
# point is the trailing V1 advance.
V1_STEPS = [1, 2, 3, 4, 6, 8, 11, 14, 17, 20, 23, 25, 26, 27, 28, 28]
V1_MAX, V2_MAX, V3_MAX = HR - 1, HR - 2, RP  # 27, 26, 24

_CACHE = {}


def _build_nc(v1_steps=None, dma_chunks=None, act_chunks=None, dve_conv=0):
    _ensure_path()
    from concourse import bacc, mybir, tile
    from concourse.ap import AP

    f32 = mybir.dt.float32
    f16 = mybir.dt.float16
    steps = list(v1_steps or V1_STEPS)
    dchunks = list(dma_chunks or DMA_CHUNKS)
    # the DVE converts rows [0, dve_conv) itself (it is idle during the
    # ramp anyway, and skipping the ACT hop cuts the feed latency); ACT
    # handles the rest.
    achunks = [
        (max(r0, dve_conv), r1)
        for r0, r1 in (act_chunks or ACT_CHUNKS)
        if r1 > dve_conv
    ]
    assert steps[-1] >= V3_MAX + 4

    nc = bacc.Bacc(
        "TRN2",
        target_bir_lowering=False,
        debug=False,
        enable_asserts=False,
        num_devices=N_CORES,
    )
    x = nc.dram_tensor("x", [128, HR, WROW], f32, kind="ExternalInput")
    y = nc.dram_tensor("y", [ROWS, WROW], f16, kind="ExternalOutput")

    with tile.TileContext(nc) as tc:
        with tc.tile_pool(name="pool", bufs=1) as pool:
            s = pool.tile([128, HR, WROW], f32, name="s", tag="s")
            w = pool.tile([128, HR, PADW], f16, name="w", tag="w")

            # warm the ACT activation table at t=0 so the implicit
            # ACT_TABLE_LOAD isn't charged to the first convert. Touches
            # only pad cols; the memsets below re-zero them.
            nc.scalar.copy(w[:, 0:1, 0:2], w[:, 0:1, 0:2])
            # zero the side pads (idle Pool engine, off the critical path)
            nc.gpsimd.memset(w[:, :, 0:PAD], 0.0)
            nc.gpsimd.memset(w[:, :, W1:PADW], 0.0)

            # ---- input DMA (uniform chunk stream, halo pre-tiled) ----
            for r0, r1 in dchunks:
                nc.sync.dma_start(
                    s[:, r0:r1, :],
                    AP(
                        x,
                        r0 * WROW,
                        [[HR * WROW, 128], [WROW, r1 - r0], [1, WROW]],
                    ),
                )

            # ---- ACT fp32 -> fp16 convert ----
            for r0, r1 in achunks:
                nc.scalar.copy(w[:, r0:r1, W0:W1], s[:, r0:r1, :])

            # ---- DVE passes + stores, software-pipelined ----
            e = nc.vector
            f1 = f2 = f3 = 0  # frontiers: rows done per pass
            fc = 0  # DVE-convert frontier (rows < dve_conv)

            def vpass(a0, a1, shift):
                if a1 > a0:
                    e.tensor_max(
                        w[:, a0:a1, W0:W1],
                        w[:, a0:a1, W0:W1],
                        w[:, a0 + shift : a1 + shift, W0:W1],
                    )

            for i, a in enumerate(steps):
                n1 = min(a, V1_MAX)
                n2 = min(max(steps[i - 1] - 1, 0), V2_MAX) if i >= 1 else 0
                n3 = min(max(steps[i - 2] - 3, 0), V3_MAX) if i >= 2 else 0
                assert n2 + 1 <= f1 or n2 <= f2, (i, n2, f1)
                assert n3 + 2 <= f2 or n3 <= f3, (i, n3, f2)
                vpass(f2, n2, 1)  # win3 over rows
                vpass(f3, n3, 2)  # win5
                if n3 > f3:
                    r0, r1 = f3, n3
                    # horizontal win2/win3/win5 (pixel stride = C = 3)
                    e.tensor_max(
                        w[:, r0:r1, 0 : PADW - 3],
                        w[:, r0:r1, 0 : PADW - 3],
                        w[:, r0:r1, 3:PADW],
                    )
                    e.tensor_max(
                        w[:, r0:r1, 0 : PADW - 6],
                        w[:, r0:r1, 0 : PADW - 6],
                        w[:, r0:r1, 3 : PADW - 3],
                    )
                    e.tensor_max(
                        w[:, r0:r1, 0:WROW],
                        w[:, r0:r1, 0:WROW],
                        w[:, r0:r1, PAD : PAD + WROW],
                    )
                    nc.sync.dma_start(
                        AP(
                            y,
                            r0 * WROW,
                            [[RP * WROW, 128], [WROW, r1 - r0], [1, WROW]],
                        ),
                        w[:, r0:r1, 0:WROW],
                    )
                need = min(n1 + 1, dve_conv)
                if need > fc:
                    # fp32 -> fp16 convert on the DVE (tensor_scalar max
                    # with 0.0: identity for data >= 0, runs in 2x_2p mode)
                    e.tensor_scalar_max(
                        w[:, fc:need, W0:W1], s[:, fc:need, :], 0.0
                    )
                    fc = need
                vpass(f1, n1, 1)  # win2 over rows (stalls on ACT, so last)
                f1, f2, f3 = n1, n2, n3
            assert (f1, f2, f3) == (V1_MAX, V2_MAX, V3_MAX)

    nc.compile()
    return nc


def _get_nc():
    if "nc" not in _CACHE:
        _CACHE["nc"] = _build_nc()
    return _CACHE["nc"]


def _row_index():
    # IDX[p, r] = input row (within a core's [3072, 1152] view) whose data
    # partition p's staging row r holds: 24p + r - 2, edge-clamped to the
    # owning image's row range (replicated edge row == SAME pad for max).
    p = np.arange(128)[:, None]
    r = np.arange(HR)[None, :]
    r_abs = RP * p + r - 2
    img_lo = (p // 16) * H
    return np.clip(r_abs, img_lo, img_lo + H - 1)


def _run(images, trace=False):
    _ensure_path()
    from concourse import bass_utils

    images = np.ascontiguousarray(np.asarray(images, dtype=np.float32))
    assert images.shape == (N_CORES * B_PER, H, W, C), images.shape
    nc = _get_nc()
    per_core = images.reshape(N_CORES, ROWS, WROW)
    idx = _row_index()
    in_maps = [
        {"x": np.ascontiguousarray(per_core[i][idx])} for i in range(N_CORES)
    ]
    res = bass_utils.run_bass_kernel_spmd(
        nc, in_maps, core_ids=list(range(N_CORES)), trace=trace
    )
    out = np.concatenate(
        [res.results[i]["y"].astype(np.float32) for i in range(N_CORES)], axis=0
    )
    out = out.reshape(N_CORES * B_PER, H, W, C)[..., None]
    return out, res


def kernel(images, k=None):
    out, _ = _run(images, trace=False)
    return out
